# revision 1
# baseline (speedup 1.0000x reference)
"""AverageNode2Vec loss on 8 Trainium2 NeuronCores.

Strategy: data-parallel over the batch (1024 rows/core); both embedding
tables are replicated in each core's HBM as one combined [2M, 128] f32
table (v rows at +1M). Each partition owns 8 batch rows laid out as 56
gather slots (8 u + 8 v + 40 neg). Ten indirect-DMA gathers (one per
sequence position l) fetch 56 rows per partition into three rotating
SBUF staging regions; the DVE folds (adds) them into one accumulator,
computing the length-masked embedding sums. Padding indices point at
table row 0, which the host zeroes, so masking is free. A short DVE/ACT
epilogue forms the u.v dots, negative-sample scores, softplus terms
(Exp+Ln, since Softplus is absent from this compiler's ACT tables) and
per-partition partial sums; the host adds 8x128x2 partials, divides by B.

HW contract notes (verified by probes in this repo):
 - indirect_dma_start needs a flat 2-dim dst AP [128, n*D]; idx [128, n]
   int32 pairs p-major: dst[p, j*D:(j+1)*D] = table[idx[p, j]].
 - CCE compute_op on the indirect path is device-fatal; plain gathers +
   DVE folds instead.
 - This walrus build allows at most ONE attached sync wait per
   instruction; raw bass with explicit wait_ge instructions sidesteps it.
 - Measured steady-state: ~74 us per iteration on 8 cores (repeat-slope
   method; ~66 us is pure gather DMA at ~306 GB/s/core of random rows).
"""
import os
import numpy as np

VOCAB = 1_000_000
D = 128
B = 8192
NEG = 5
L = 10
NCORES = 8
BPC = B // NCORES            # batch rows per core
T = BPC // 128               # tile slots per partition (8)
SLOTS = T + T + T * NEG      # 56 gather slots per partition
OOB = 2 * VOCAB + 64         # skipped by bounds check

_STATE = {}
LAST_EXEC_NS = None


def _build_program(repeat=1, bf16=False):
    import concourse.bass as bass
    import concourse.mybir as mybir
    from concourse.bass import AP, IndirectOffsetOnAxis

    f32, i32 = mybir.dt.float32, mybir.dt.int32
    tdt = mybir.dt.bfloat16 if bf16 else f32
    nc = bass.Bass()
    big = nc.dram_tensor("big", [2 * VOCAB, D], tdt, kind="ExternalInput")
    idx_in = nc.dram_tensor("idx", [128, L * SLOTS], i32, kind="ExternalInput")
    ruv_in = nc.dram_tensor("ruv", [128, T], f32, kind="ExternalInput")
    rn_in = nc.dram_tensor("rn", [128, T * NEG], f32, kind="ExternalInput")
    out = nc.dram_tensor("lp", [128, 2], f32, kind="ExternalOutput")

    NU = T * D               # 1024: u block columns in acc
    NN = T * NEG * D         # 5120: neg block columns

    from contextlib import ExitStack
    ctx = ExitStack()
    with ctx:
        idx_t = ctx.enter_context(nc.sbuf_tensor([128, L * SLOTS], i32))
        g_a = ctx.enter_context(nc.sbuf_tensor([128, SLOTS * D], tdt))
        g_b = ctx.enter_context(nc.sbuf_tensor([128, SLOTS * D], tdt))
        g_c = ctx.enter_context(nc.sbuf_tensor([128, SLOTS * D], tdt))
        acc = ctx.enter_context(nc.sbuf_tensor([128, SLOTS * D], tdt))
        ruv_t = ctx.enter_context(nc.sbuf_tensor([128, T], f32))
        rn_t = ctx.enter_context(nc.sbuf_tensor([128, T * NEG], f32))
        prod_uv = ctx.enter_context(nc.sbuf_tensor([128, NU], f32))
        prod_nu = ctx.enter_context(nc.sbuf_tensor([128, NN], f32))
        score_raw = ctx.enter_context(nc.sbuf_tensor([128, T], f32))
        nscore_raw = ctx.enter_context(nc.sbuf_tensor([128, T * NEG], f32))
        score = ctx.enter_context(nc.sbuf_tensor([128, T], f32))
        nscore = ctx.enter_context(nc.sbuf_tensor([128, T * NEG], f32))
        plt_t = ctx.enter_context(nc.sbuf_tensor([128, T], f32))
        nlt_t = ctx.enter_context(nc.sbuf_tensor([128, T * NEG], f32))
        lp_t = ctx.enter_context(nc.sbuf_tensor([128, 2], f32))
        s_idx = ctx.enter_context(nc.semaphore("s_idx"))
        s_rcp = ctx.enter_context(nc.semaphore("s_rcp"))
        s_g = ctx.enter_context(nc.semaphore("s_g"))
        s_f = ctx.enter_context(nc.semaphore("s_f"))
        s_dve = ctx.enter_context(nc.semaphore("s_dve"))
        s_act = ctx.enter_context(nc.semaphore("s_act"))
        s_out = ctx.enter_context(nc.semaphore("s_out"))
        block = ctx.enter_context(nc.Block())

        @block.sync
        def _(sync):
            sync.dma_start(out=idx_t[:], in_=idx_in[:]).then_inc(s_idx, 16)
            sync.dma_start(out=ruv_t[:], in_=ruv_in[:]).then_inc(s_rcp, 16)
            sync.dma_start(out=rn_t[:], in_=rn_in[:]).then_inc(s_rcp, 16)
            for r in range(repeat):
                sync.wait_ge(s_act, 2 * (r + 1))
                sync.dma_start(out=out[:], in_=lp_t[:]).then_inc(s_out, 16)

        regions = [g_a, g_b, g_c]

        @block.gpsimd
        def _(gpsimd):
            gpsimd.wait_ge(s_idx, 16)
            for r in range(repeat):
                for l in range(L):
                    # wait until the previous occupant of region l%3 has been
                    # consumed by its fold (see fold numbering below)
                    if r == 0:
                        tgt = max(1, l - 3) if l >= 3 else 0
                    else:
                        tgt = 9 * r + max(1, l - 3) if l >= 3 else (
                            9 * r if l == 0 else 9 * r - 3 + l)
                    if tgt > 0:
                        gpsimd.wait_ge(s_f, tgt)
                    gpsimd.indirect_dma_start(
                        out=regions[l % 3][:],
                        out_offset=None,
                        in_=big[:],
                        in_offset=IndirectOffsetOnAxis(
                            ap=idx_t[:, l * SLOTS:(l + 1) * SLOTS], axis=0
                        ),
                    ).then_inc(s_g, 16)

        def _epilogue(vector, r):
            a = acc[:]
            acc_u = acc[:, 0:NU]
            acc_v = acc[:, NU:2 * NU]
            acc_n = acc[:, 2 * NU:]
            vector.tensor_tensor(
                out=prod_uv[:], in0=acc_u, in1=acc_v, op=mybir.AluOpType.mult
            )
            vector.tensor_reduce(
                out=score_raw[:],
                in_=prod_uv[:].rearrange("p (t d) -> p t d", d=D),
                axis=mybir.AxisListType.X,
                op=mybir.AluOpType.add,
            )
            # acc_u broadcast across the NEG axis: [p, t, n(bcast), d]
            acc_u_b = AP(
                a.tensor, a.offset,
                [[a.ap[0][0], 128], [D, T], [0, NEG], [1, D]],
            )
            vector.tensor_tensor(
                out=prod_nu[:],
                in0=acc_n.rearrange("p (t n d) -> p t n d", n=NEG, d=D),
                in1=acc_u_b,
                op=mybir.AluOpType.mult,
            )
            vector.tensor_reduce(
                out=nscore_raw[:],
                in_=prod_nu[:].rearrange("p (s d) -> p s d", d=D),
                axis=mybir.AxisListType.X,
                op=mybir.AluOpType.add,
            )
            if r == 0:
                vector.wait_ge(s_rcp, 32)
            if r > 0:
                # ACT of rep r-1 must have consumed score/nscore
                vector.wait_ge(s_act, 2 * r)
            vector.tensor_tensor(
                out=score[:], in0=score_raw[:], in1=ruv_t[:],
                op=mybir.AluOpType.mult,
            ).then_inc(s_dve, 1)
            vector.tensor_tensor(
                out=nscore[:], in0=nscore_raw[:], in1=rn_t[:],
                op=mybir.AluOpType.mult,
            ).then_inc(s_dve, 1)

        @block.vector
        def _(vector):
            for r in range(repeat):
                base_g = 160 * r
                # fold 1: acc = g0 + g1; fold k (2..9): acc += region(k % 3)
                vector.wait_ge(s_g, base_g + 32)
                vector.tensor_tensor(
                    out=acc[:], in0=g_a[:], in1=g_b[:], op=mybir.AluOpType.add
                ).then_inc(s_f, 1)
                for k in range(2, L):
                    vector.wait_ge(s_g, base_g + 16 * (k + 1))
                    vector.tensor_tensor(
                        out=acc[:], in0=acc[:], in1=regions[k % 3][:],
                        op=mybir.AluOpType.add,
                    ).then_inc(s_f, 1)
                _epilogue(vector, r)

        @block.scalar
        def _(scalar):
            # softplus(x) = ln(1 + exp(x)); Softplus itself is absent from
            # this compiler's ACT table set, Exp/Ln are present.
            for r in range(repeat):
                scalar.wait_ge(s_dve, 2 * (r + 1))
                if r > 0:
                    # out-DMA of rep r-1 must have read lp_t
                    scalar.wait_ge(s_out, 16 * r)
                scalar.activation(
                    out=plt_t[:], in_=score[:],
                    func=mybir.ActivationFunctionType.Exp, scale=-1.0,
                )
                scalar.activation(
                    out=plt_t[:], in_=plt_t[:],
                    func=mybir.ActivationFunctionType.Ln,
                    bias=1.0, accum_out=lp_t[:, 0:1],
                ).then_inc(s_act, 1)
                scalar.activation(
                    out=nlt_t[:], in_=nscore[:],
                    func=mybir.ActivationFunctionType.Exp, scale=1.0,
                )
                scalar.activation(
                    out=nlt_t[:], in_=nlt_t[:],
                    func=mybir.ActivationFunctionType.Ln,
                    bias=1.0, accum_out=lp_t[:, 1:2],
                ).then_inc(s_act, 1)

    return nc


def _prep_core_inputs(c, pos_u, pos_v, neg_v, lu_all, lv_all, ln_all):
    """Index tile [128, L*SLOTS] + recip tiles for core c."""
    bsel = slice(c * BPC, (c + 1) * BPC)
    nsel = slice(c * BPC * NEG, (c + 1) * BPC * NEG)
    pu = pos_u[bsel].reshape(T, 128, L).transpose(1, 2, 0)          # [p, l, t]
    pv = pos_v[bsel].reshape(T, 128, L).transpose(1, 2, 0)          # [p, l, t]
    nv = neg_v[nsel].reshape(T, 128, NEG, L).transpose(1, 3, 0, 2)  # [p, l, t, n]
    nv = nv.reshape(128, L, T * NEG)

    idx = np.empty((128, L, SLOTS), np.int64)
    idx[:, :, 0:T] = pu
    idx[:, :, T:2 * T] = np.where(pv == 0, 0, pv + VOCAB)
    idx[:, :, 2 * T:] = np.where(nv == 0, 0, nv + VOCAB)
    # padding (index 0) gathers the zeroed row 0 and contributes nothing
    idx_flat = np.ascontiguousarray(idx.reshape(128, L * SLOTS), dtype=np.int32)

    lu = lu_all[bsel].reshape(T, 128).T.astype(np.float64)           # [p, t]
    lv = lv_all[bsel].reshape(T, 128).T.astype(np.float64)
    ln = ln_all[nsel].reshape(T, 128, NEG).transpose(1, 0, 2).astype(np.float64)
    ruv = (1.0 / (lu * lv)).astype(np.float32)
    rn = (1.0 / (lu[:, :, None] * ln)).reshape(128, T * NEG).astype(np.float32)
    return idx_flat, np.ascontiguousarray(ruv), np.ascontiguousarray(rn)


def kernel(u_table, v_table, pos_u, pos_u_lens, pos_v, pos_v_lens,
           neg_v, neg_v_lens):
    global LAST_EXEC_NS
    from concourse.bass_utils import run_bass_kernel_spmd

    u_table = np.asarray(u_table)
    v_table = np.asarray(v_table)
    pos_u = np.asarray(pos_u)
    pos_v = np.asarray(pos_v)
    neg_v = np.asarray(neg_v)
    pos_u_lens = np.asarray(pos_u_lens)
    pos_v_lens = np.asarray(pos_v_lens)
    neg_v_lens = np.asarray(neg_v_lens)

    bf16 = bool(int(os.environ.get("KERNEL_BF16", "0")))
    big = np.empty((2 * VOCAB, D), np.float32)
    big[:VOCAB] = u_table
    big[VOCAB:] = v_table
    big[0] = 0.0          # padding rows (index 0) must contribute zero
    big[VOCAB] = 0.0
    if bf16:
        import ml_dtypes
        big = big.astype(ml_dtypes.bfloat16)

    key = ("nc", bf16)
    if key not in _STATE:
        _STATE[key] = _build_program(bf16=bf16)
    nc = _STATE[key]

    in_maps = []
    for c in range(NCORES):
        idx, ruv, rn = _prep_core_inputs(
            c, pos_u, pos_v, neg_v, pos_u_lens, pos_v_lens, neg_v_lens
        )
        in_maps.append({"big": big, "idx": idx, "ruv": ruv, "rn": rn})

    trace = bool(int(os.environ.get("KERNEL_TRACE", "0")))
    res = run_bass_kernel_spmd(
        nc, in_maps, core_ids=list(range(NCORES)), trace=trace,
    )
    LAST_EXEC_NS = res.exec_time_ns

    total = np.float64(0.0)
    for r in res.results:
        total += r["lp"].astype(np.float64).sum()
    return np.float32(total / B)

